# revision 37
# baseline (speedup 1.0000x reference)
"""Trainium2 Bass kernel for masked Sinkhorn (BiStochastic) normalization.

Reference computation (per sample b of B=128):
    mask = (i < nrows[b]) & (j < ncols[b])       on a [512, 512] score matrix
    log_s = where(mask, s, -1e30)
    10 alternating logsumexp normalizations (row, col, row, ...)
    out = where(mask, exp(log_s), 0)

Kernel strategy (exp-space Sinkhorn, numerically equivalent to ~1e-5):
    P0 = exp(s + rowbias) * colmask          (rowbias per-partition via ACT bias)
    then 5 pairs of (row-normalize, col-normalize), each pair fused into ONE
    elementwise pass per tile:
        P <- (P * rowrecip) * colrecip_bcast     [scalar_tensor_tensor]
    with the NEXT row-sum produced for free by the fused accumulator
    (accum_out), the weighted column-sums computed on the PE as fp32 matmuls
    with the row-reciprocal vector as the stationary operand (4 sample slots
    share a PSUM bank at partition offsets 0/32/64/96 via tile_position), and
    the column-reciprocal broadcast materialized by a one-hot row-select
    matmul (identity column x [4,M] vector tile) into PSUM.

Input adaptivity: only the valid region of each sample (ceil(nrows/128) row
tiles x pad32(ncols) columns) is loaded and computed; the rest of the output
stays zero.  Samples are sorted by work and snake-dealt across the 8 cores so
one SPMD program's per-slot trip counts fit all cores with balanced work.
The compiled module is cached per slot geometry.

Sharding: data parallel over the batch dim, 16 sample slots per core x 8.
"""

import os
import sys
import numpy as np

sys.path.insert(0, "/opt/trn_rl_repo")

from contextlib import ExitStack

import concourse.bass as bass
import concourse.bacc as bacc
import concourse.tile as tile
from concourse import mybir
from concourse.bass_utils import run_bass_kernel_spmd

B, N, M = 128, 512, 512
NCORES = 8
BPC = B // NCORES          # sample slots per core = 16
P = 128                    # SBUF partitions
NT = N // P                # max row tiles per sample = 4
NPAIRS = 5                 # 10 normalizations = 5 (row, col) pairs
TINY = 1e-30
NEG = -1e30
G = 8                      # lockstep slot-group size (2 groups of 8)
NG = BPC // G

F32 = mybir.dt.float32
BF16 = mybir.dt.bfloat16

LAST_EXEC_NS = None
_CACHE = {}
CB_MODE = os.environ.get("BISTOCH_CB", "dma")  # "pe" | "dma"


def plan_from_masks(nrows, ncols):
    """Sort samples by work, snake-deal across cores, compute slot geometry."""
    nrows = np.asarray(nrows).astype(np.int64)
    ncols = np.asarray(ncols).astype(np.int64)
    vt = np.minimum(np.maximum((nrows + P - 1) // P, 0), NT)   # valid row tiles
    ncp = np.minimum((np.maximum(ncols, 0) + 31) // 32 * 32, M)  # padded cols
    vt = np.where(ncp == 0, 0, vt)
    ncp = np.where(vt == 0, 0, ncp)
    # lexicographic (vt, ncp) descending keeps slot max-dims tight
    order = np.lexsort((-ncp, -vt))
    sample_at = np.zeros((NCORES, BPC), dtype=np.int64)
    for k in range(BPC):
        ranks = order[k * NCORES : (k + 1) * NCORES]
        if k % 2 == 1:
            ranks = ranks[::-1]
        for c in range(NCORES):
            sample_at[c, k] = ranks[c]
    VT = [int(vt[sample_at[:, k]].max()) for k in range(BPC)]
    NC = [int(ncp[sample_at[:, k]].max()) for k in range(BPC)]
    for k in range(BPC):
        if VT[k] == 0 or NC[k] == 0:
            VT[k], NC[k] = 0, 0
    # fused-pass engine routing: send ~1/3 of the work to POOL (2x slower
    # than DVE for 2-input fp32), choosing whole slots, smallest first
    # POOL does not implement TensorScalarPtr on TRN2 (engine check fails
    # in codegen) -- run every fused pass on DVE.
    pool_slots = []
    return {
        "sample_at": sample_at,
        "VT": tuple(VT),
        "NC": tuple(NC),
        "pool_slots": tuple(sorted(pool_slots)),
    }


def build(plan):
    VT, NC = plan["VT"], plan["NC"]
    pool_slots = set(plan["pool_slots"])
    sum_vt = sum(VT)
    # rs free-dim offsets (group-local) and rowbias offsets (global)
    off = [0] * BPC
    goff = [0] * BPC
    for k in range(BPC):
        kk = k % G
        off[k] = 0 if kk == 0 else off[k - 1] + VT[k - 1]
        goff[k] = 0 if k == 0 else goff[k - 1] + VT[k - 1]
    gvt = [sum(VT[g * G : (g + 1) * G]) for g in range(NG)]

    nc = bacc.Bacc()
    s_d = nc.declare_dram_parameter("s", [BPC, N, M], F32, isOutput=False)
    rb_d = nc.declare_dram_parameter("rowbias", [P, max(sum_vt, 1)], F32,
                                     isOutput=False)
    cm_d = nc.declare_dram_parameter("colmask", [BPC, M], F32, isOutput=False)
    out_d = nc.declare_dram_parameter("out", [BPC, N, M], F32, isOutput=True)

    s_all = s_d[:].rearrange("b (t p) j -> b p t j", p=P)
    out_all = out_d[:].rearrange("b (t p) j -> b p t j", p=P)

    with tile.TileContext(nc) as tc, ExitStack() as ctx:
        consts = ctx.enter_context(tc.tile_pool(name="consts", bufs=1))
        rb_sb = consts.tile([P, max(sum_vt, 1)], F32)
        nc.sync.dma_start(out=rb_sb, in_=rb_d[:])
        cm_sb = consts.tile([BPC, M], F32)
        nc.sync.dma_start(out=cm_sb, in_=cm_d[:])
        tiny1 = consts.tile([1, 1], BF16)
        nc.vector.memset(tiny1, TINY)
        ones512 = consts.tile([1, M], BF16)
        nc.vector.memset(ones512, 1.0)
        # consume the rowbias DMA on ACT once: later exps then carry only
        # their own s-tile DMA wait (ACT encodes at most one DMA-sem wait)
        warm = consts.tile([P, 1], F32)
        nc.scalar.copy(out=warm, in_=rb_sb[:, 0:1])

        pool_P = ctx.enter_context(tc.tile_pool(name="pmat", bufs=1))
        pool_rs = ctx.enter_context(tc.tile_pool(name="rs", bufs=2))
        pool_sm = ctx.enter_context(tc.tile_pool(name="smalls", bufs=2))
        pool_cbs = ctx.enter_context(tc.tile_pool(name="cbsb", bufs=8))
        n_cs_bufs = 7 if CB_MODE == "dma" else 4
        psum_cb = ctx.enter_context(tc.tile_pool(name="cbps", bufs=3, space="PSUM"))
        psum_cs = ctx.enter_context(
            tc.tile_pool(name="cs", bufs=n_cs_bufs, space="PSUM"))
        pool_crd = ctx.enter_context(tc.tile_pool(name="crd", bufs=4, space="DRAM"))

        def fused(k, t, rr_scalar, cb, acc):
            # P[:, t, :ncw] = (P * rr) * cb ; acc = rowsum of result
            ncw = NC[k]
            pt = Pt[k]
            eng = nc.gpsimd if k in pool_slots else nc.vector
            eng.scalar_tensor_tensor(
                out=pt[:, t, :ncw], in0=pt[:, t, :ncw],
                scalar=rr_scalar, in1=cb[:, :ncw],
                op0=mybir.AluOpType.mult, op1=mybir.AluOpType.mult,
                accum_out=acc,
            )

        ident = None
        if CB_MODE == "pe":
            ident_d = nc.inline_tensor(np.eye(BPC, dtype=np.float32),
                                       name="ident")
            ident = consts.tile([BPC, BPC], F32)
            nc.sync.dma_start(out=ident, in_=ident_d[:])

        def make_cb(k, dram_vec, sbuf_vec, nk, col):
            # cb[i, j] = vec[col, j] broadcast over partitions
            ncw = NC[k]
            if CB_MODE == "pe" and sbuf_vec is not None:
                # one-hot row-select matmul into PSUM, then ACT copy to SBUF
                # for POOL consumers (POOL cannot read PSUM)
                cbp = psum_cb.tile([P, M], F32, tag="cbp", name=f"cbp_{k}")
                nc.tensor.matmul(
                    out=cbp[:, :ncw],
                    lhsT=ident[:nk, col : col + 1].broadcast_to([nk, P]),
                    rhs=sbuf_vec[:, :ncw], start=True, stop=True,
                )
                if k in pool_slots:
                    cbs = pool_cbs.tile([P, M], F32, tag="cbsb",
                                        name=f"cb_{k}")
                    nc.scalar.copy(out=cbs[:, :ncw], in_=cbp[:, :ncw])
                    return cbs
                return cbp
            cb = pool_cbs.tile([P, M], F32, tag="cbsb", name=f"cb_{k}")
            nc.gpsimd.dma_start(
                out=cb[:, :ncw],
                in_=dram_vec[col : col + 1, :ncw].to_broadcast([P, ncw]),
            )
            return cb

        Pt = [None] * BPC
        rs_cur = [None] * NG

        for g in range(NG):
            if gvt[g] == 0:
                continue
            rs0 = pool_rs.tile([P, gvt[g]], F32, tag=f"rs{g}")
            rs_cur[g] = rs0
            for kk in range(G):
                k = g * G + kk
                if VT[k] == 0:
                    continue
                vt, ncw = VT[k], NC[k]
                pt = pool_P.tile([P, vt, M], F32, tag=f"pmat{k}")
                Pt[k] = pt
                if ncw < M:
                    nc.gpsimd.memset(pt[:, :, ncw:], 0.0)
                # per-tile loads keep each exp's semaphore fan-in small
                # (one big DMA splits across queues -> too many sync waits)
                for t in range(vt):
                    nc.sync.dma_start(out=pt[:, t, :ncw],
                                      in_=s_all[k][:, t, :ncw])
                # P0 = exp(s + rowbias): rowbias = 0 on valid rows, -1e30 off
                for t in range(vt):
                    nc.scalar.activation(
                        out=pt[:, t, :ncw], in_=pt[:, t, :ncw],
                        func=mybir.ActivationFunctionType.Exp,
                        bias=rb_sb[:, goff[k] + t : goff[k] + t + 1],
                        scale=1.0,
                    )
                # init: P0 *= colmask (bf16 one-hot broadcast is exact for
                # a 0/1 mask), with fused row-sum accumulation
                cb = make_cb(k, cm_d[:], cm_sb, BPC, k)
                for t in range(vt):
                    fused(k, t, 1.0, cb,
                          rs0[:, off[k] + t : off[k] + t + 1])

        for pair in range(NPAIRS):
            last = pair == NPAIRS - 1
            rs_nx = [None] * NG
            for g in range(NG):
                if rs_cur[g] is None:
                    continue
                # row reciprocals for the whole group: rr = 1/(rs + tiny)
                tmp = pool_sm.tile([P, gvt[g]], F32, tag=f"tmp{g}")
                rr = pool_sm.tile([P, gvt[g]], F32, tag=f"rr{g}")
                nc.vector.tensor_scalar_add(tmp, rs_cur[g], TINY)
                nc.vector.reciprocal(rr, tmp)

                if not last:
                    rs_nx[g] = pool_rs.tile([P, gvt[g]], F32, tag=f"rs{g}",
                                            name=f"rsn{pair}_{g}")

                for q4 in range(G // 4):
                    slots = [g * G + q4 * 4 + i for i in range(4)]
                    slots = [k for k in slots if Pt[k] is not None]
                    if not slots:
                        continue
                    # weighted column sums: cs[q, j] = sum_i rr[i] * P[i, j],
                    # 4 slots per PSUM bank at partition offsets 0/32/64/96,
                    # TINY-seeded so fully-masked columns stay finite.
                    csb = psum_cs.tile([P, M], F32, tag="cs")
                    for k in slots:
                        q = k % 4
                        region = csb[32 * q : 32 * q + 1, :]
                        nc.tensor.matmul(
                            out=region[:, : NC[k]], lhsT=tiny1,
                            rhs=ones512[:, : NC[k]],
                            start=True, stop=False, tile_position=(0, 32 * q),
                        )
                        for t in range(VT[k]):
                            nc.tensor.matmul(
                                out=region[:, : NC[k]],
                                lhsT=rr[:, off[k] + t : off[k] + t + 1],
                                rhs=Pt[k][:, t, : NC[k]],
                                start=False, stop=(t == VT[k] - 1),
                                tile_position=(0, 32 * q),
                            )
                    for k in slots:
                        q = k % 4
                        ncw = NC[k]
                        # per-slot single-partition recip: a PSUM read at
                        # partition offset 32q is legal (DVE cannot read
                        # strided-partition views); out lands at partition 0
                        cr1 = pool_sm.tile([1, M], F32, tag=f"cr_{k}",
                                           name=f"cr{pair}_{k}")
                        nc.vector.reciprocal(
                            cr1[:, :ncw], csb[32 * q : 32 * q + 1, :ncw])
                        crd1 = None
                        if CB_MODE == "dma":
                            crd1 = pool_crd.tile([1, M], F32, tag="crd",
                                                 name=f"crd{pair}_{k}")
                            nc.sync.dma_start(out=crd1[:, :ncw],
                                              in_=cr1[:, :ncw])
                        cb = make_cb(k, crd1, cr1, 1, 0)
                        for t in range(VT[k]):
                            acc = None
                            if not last:
                                acc = rs_nx[g][:, off[k] + t : off[k] + t + 1]
                            fused(k, t, rr[:, off[k] + t : off[k] + t + 1],
                                  cb, acc)
            rs_cur = rs_nx

        for k in range(BPC):
            if Pt[k] is not None:
                nc.sync.dma_start(out=out_all[k][:, : VT[k], :], in_=Pt[k])

    # Bacc legalization: splits multi-semaphore waits (TRN2 allows one wait
    # per compute instruction), event-semaphore generation, reg alloc, DCE.
    nc.compile()
    return nc


def host_inputs(plan, s, nrows, ncols):
    """Per-core input maps (permuted into slot order) + mask vectors."""
    s = np.ascontiguousarray(np.asarray(s), dtype=np.float32)
    nrows = np.asarray(nrows).astype(np.int64)
    ncols = np.asarray(ncols).astype(np.int64)
    VT, NC = plan["VT"], plan["NC"]
    sample_at = plan["sample_at"]
    sum_vt = max(sum(VT), 1)
    in_maps = []
    for c in range(NCORES):
        idx = sample_at[c]
        s_c = s[idx]
        rb = np.full((P, sum_vt), 0.0, dtype=np.float32)
        pos = 0
        for k in range(BPC):
            nr = nrows[idx[k]]
            for t in range(VT[k]):
                gi = t * P + np.arange(P)
                rb[:, pos] = np.where(gi < nr, 0.0, NEG)
                pos += 1
        cm = (np.arange(M)[None, :] < ncols[idx][:, None]).astype(np.float32)
        in_maps.append(
            {
                "s": np.ascontiguousarray(s_c),
                "rowbias": rb,
                "colmask": np.ascontiguousarray(cm),
            }
        )
    return in_maps


def kernel(s, nrows, ncols):
    global LAST_EXEC_NS
    plan = plan_from_masks(nrows, ncols)
    key = (plan["VT"], plan["NC"], plan["pool_slots"])
    if key not in _CACHE:
        _CACHE[key] = build(plan)
    nc = _CACHE[key]
    in_maps = host_inputs(plan, s, nrows, ncols)
    res = run_bass_kernel_spmd(nc, in_maps, list(range(NCORES)))
    LAST_EXEC_NS = res.exec_time_ns
    out = np.zeros((B, N, M), dtype=np.float32)
    sample_at = plan["sample_at"]
    for c in range(NCORES):
        out[sample_at[c]] = res.results[c]["out"]
    return out


# revision 46
# speedup vs baseline: 1.3338x; 1.3338x over previous
"""Trainium2 Bass kernel for masked Sinkhorn (BiStochastic) normalization.

Reference computation (per sample b of B=128):
    mask = (i < nrows[b]) & (j < ncols[b])       on a [512, 512] score matrix
    log_s = where(mask, s, -1e30)
    10 alternating logsumexp normalizations (row, col, row, ...)
    out = where(mask, exp(log_s), 0)

Kernel strategy (exp-space Sinkhorn, numerically equivalent to ~1e-5):
    P0 = exp(s + rowbias) * colmask          (rowbias per-partition via ACT bias)
    then 5 pairs of (row-normalize, col-normalize), each pair fused into ONE
    elementwise pass per tile:
        P <- (P * rowrecip) * colrecip_bcast     [scalar_tensor_tensor]
    with the NEXT row-sum produced for free by the fused accumulator
    (accum_out), the weighted column-sums computed on the PE as fp32 matmuls
    with the row-reciprocal vector as the stationary operand (4 sample slots
    share a PSUM bank at partition offsets 0/32/64/96 via tile_position), and
    the column-reciprocal broadcast materialized by a one-hot row-select
    matmul (identity column x [4,M] vector tile) into PSUM.

Input adaptivity: only the valid region of each sample (ceil(nrows/128) row
tiles x pad32(ncols) columns) is loaded and computed; the rest of the output
stays zero.  Samples are sorted by work and snake-dealt across the 8 cores so
one SPMD program's per-slot trip counts fit all cores with balanced work.
The compiled module is cached per slot geometry.

Sharding: data parallel over the batch dim, 16 sample slots per core x 8.
"""

import os
import sys
import numpy as np

sys.path.insert(0, "/opt/trn_rl_repo")

from contextlib import ExitStack

import concourse.bass as bass
import concourse.bacc as bacc
import concourse.tile as tile
from concourse import mybir
from concourse.bass_utils import run_bass_kernel_spmd

B, N, M = 128, 512, 512
NCORES = 8
BPC = B // NCORES          # sample slots per core = 16
P = 128                    # SBUF partitions
NT = N // P                # max row tiles per sample = 4
NPAIRS = 5                 # 10 normalizations = 5 (row, col) pairs
TINY = 1e-30
NEG = -1e30
G = 4                      # lockstep slot-group size (4 groups of 4)
NG = BPC // G

F32 = mybir.dt.float32
BF16 = mybir.dt.bfloat16

LAST_EXEC_NS = None
_CACHE = {}
CB_MODE = os.environ.get("BISTOCH_CB", "dma")  # "pe" | "dma"


def plan_from_masks(nrows, ncols):
    """Sort samples by work, snake-deal across cores, compute slot geometry."""
    nrows = np.asarray(nrows).astype(np.int64)
    ncols = np.asarray(ncols).astype(np.int64)
    vt = np.minimum(np.maximum((nrows + P - 1) // P, 0), NT)   # valid row tiles
    ncp = np.minimum((np.maximum(ncols, 0) + 31) // 32 * 32, M)  # padded cols
    vt = np.where(ncp == 0, 0, vt)
    ncp = np.where(vt == 0, 0, ncp)
    # lexicographic (vt, ncp) descending keeps slot max-dims tight
    order = np.lexsort((-ncp, -vt))
    sample_at = np.zeros((NCORES, BPC), dtype=np.int64)
    for k in range(BPC):
        ranks = order[k * NCORES : (k + 1) * NCORES]
        if k % 2 == 1:
            ranks = ranks[::-1]
        for c in range(NCORES):
            sample_at[c, k] = ranks[c]
    VT = [int(vt[sample_at[:, k]].max()) for k in range(BPC)]
    NC = [int(ncp[sample_at[:, k]].max()) for k in range(BPC)]
    for k in range(BPC):
        if VT[k] == 0 or NC[k] == 0:
            VT[k], NC[k] = 0, 0
    # fused-pass engine routing: send ~1/3 of the work to POOL (2x slower
    # than DVE for 2-input fp32), choosing whole slots, smallest first
    # POOL does not implement TensorScalarPtr on TRN2 (engine check fails
    # in codegen) -- run every fused pass on DVE.
    pool_slots = []
    return {
        "sample_at": sample_at,
        "VT": tuple(VT),
        "NC": tuple(NC),
        "pool_slots": tuple(sorted(pool_slots)),
    }


def build(plan):
    VT, NC = plan["VT"], plan["NC"]
    pool_slots = set(plan["pool_slots"])
    sum_vt = sum(VT)
    # rs free-dim offsets (group-local) and rowbias offsets (global)
    off = [0] * BPC
    goff = [0] * BPC
    for k in range(BPC):
        kk = k % G
        off[k] = 0 if kk == 0 else off[k - 1] + VT[k - 1]
        goff[k] = 0 if k == 0 else goff[k - 1] + VT[k - 1]
    gvt = [sum(VT[g * G : (g + 1) * G]) for g in range(NG)]

    nc = bacc.Bacc()
    s_d = nc.declare_dram_parameter("s", [BPC, N, M], F32, isOutput=False)
    rb_d = nc.declare_dram_parameter("rowbias", [P, max(sum_vt, 1)], F32,
                                     isOutput=False)
    cm_d = nc.declare_dram_parameter("colmask", [BPC, M], F32, isOutput=False)
    out_d = nc.declare_dram_parameter("out", [BPC, N, M], F32, isOutput=True)

    s_all = s_d[:].rearrange("b (t p) j -> b p t j", p=P)
    out_all = out_d[:].rearrange("b (t p) j -> b p t j", p=P)

    with tile.TileContext(nc) as tc, ExitStack() as ctx:
        consts = ctx.enter_context(tc.tile_pool(name="consts", bufs=1))
        rb_sb = consts.tile([P, max(sum_vt, 1)], F32)
        nc.sync.dma_start(out=rb_sb, in_=rb_d[:])
        cm_sb = consts.tile([BPC, M], F32)
        nc.sync.dma_start(out=cm_sb, in_=cm_d[:])
        tiny1 = consts.tile([1, 1], BF16)
        nc.vector.memset(tiny1, TINY)
        ones512 = consts.tile([1, M], BF16)
        nc.vector.memset(ones512, 1.0)
        # consume the rowbias DMA on ACT once: later exps then carry only
        # their own s-tile DMA wait (ACT encodes at most one DMA-sem wait)
        warm = consts.tile([P, 1], F32)
        nc.scalar.copy(out=warm, in_=rb_sb[:, 0:1])

        pool_P = ctx.enter_context(tc.tile_pool(name="pmat", bufs=1))
        pool_rs = ctx.enter_context(tc.tile_pool(name="rs", bufs=2))
        pool_sm = ctx.enter_context(tc.tile_pool(name="smalls", bufs=2))
        pool_cbs = ctx.enter_context(tc.tile_pool(name="cbsb", bufs=8))
        n_cs_bufs = 7 if CB_MODE == "dma" else 4
        psum_cb = ctx.enter_context(tc.tile_pool(name="cbps", bufs=3, space="PSUM"))
        psum_cs = ctx.enter_context(
            tc.tile_pool(name="cs", bufs=n_cs_bufs, space="PSUM"))
        pool_crd = ctx.enter_context(tc.tile_pool(name="crd", bufs=4, space="DRAM"))

        def fused(k, t, rr_scalar, cb, acc):
            # P[:, t, :ncw] = (P * rr) * cb ; acc = rowsum of result
            ncw = NC[k]
            pt = Pt[k]
            eng = nc.gpsimd if k in pool_slots else nc.vector
            eng.scalar_tensor_tensor(
                out=pt[:, t, :ncw], in0=pt[:, t, :ncw],
                scalar=rr_scalar, in1=cb[:, :ncw],
                op0=mybir.AluOpType.mult, op1=mybir.AluOpType.mult,
                accum_out=acc,
            )

        ident = None
        if CB_MODE == "pe":
            ident_d = nc.inline_tensor(np.eye(BPC, dtype=np.float32),
                                       name="ident")
            ident = consts.tile([BPC, BPC], F32)
            nc.sync.dma_start(out=ident, in_=ident_d[:])

        def make_cb(k, dram_vec, sbuf_vec, nk, col):
            # cb[i, j] = vec[col, j] broadcast over partitions
            ncw = NC[k]
            if CB_MODE == "pe" and sbuf_vec is not None:
                # one-hot row-select matmul into PSUM, then ACT copy to SBUF
                # for POOL consumers (POOL cannot read PSUM)
                cbp = psum_cb.tile([P, M], F32, tag="cbp", name=f"cbp_{k}")
                nc.tensor.matmul(
                    out=cbp[:, :ncw],
                    lhsT=ident[:nk, col : col + 1].broadcast_to([nk, P]),
                    rhs=sbuf_vec[:, :ncw], start=True, stop=True,
                )
                if k in pool_slots:
                    cbs = pool_cbs.tile([P, M], F32, tag="cbsb",
                                        name=f"cb_{k}")
                    nc.scalar.copy(out=cbs[:, :ncw], in_=cbp[:, :ncw])
                    return cbs
                return cbp
            cb = pool_cbs.tile([P, M], F32, tag="cbsb", name=f"cb_{k}")
            nc.gpsimd.dma_start(
                out=cb[:, :ncw],
                in_=dram_vec[col : col + 1, :ncw].to_broadcast([P, ncw]),
            )
            return cb

        Pt = [None] * BPC
        rs_cur = [None] * NG

        for g in range(NG):
            if gvt[g] == 0:
                continue
            rs0 = pool_rs.tile([P, gvt[g]], F32, tag=f"rs{g}")
            rs_cur[g] = rs0
            for kk in range(G):
                k = g * G + kk
                if VT[k] == 0:
                    continue
                vt, ncw = VT[k], NC[k]
                pt = pool_P.tile([P, vt, M], F32, tag=f"pmat{k}")
                Pt[k] = pt
                if ncw < M:
                    nc.gpsimd.memset(pt[:, :, ncw:], 0.0)
                # per-tile loads keep each exp's semaphore fan-in small
                # (one big DMA splits across queues -> too many sync waits)
                for t in range(vt):
                    nc.sync.dma_start(out=pt[:, t, :ncw],
                                      in_=s_all[k][:, t, :ncw])
                # P0 = exp(s + rowbias): rowbias = 0 on valid rows, -1e30 off
                for t in range(vt):
                    nc.scalar.activation(
                        out=pt[:, t, :ncw], in_=pt[:, t, :ncw],
                        func=mybir.ActivationFunctionType.Exp,
                        bias=rb_sb[:, goff[k] + t : goff[k] + t + 1],
                        scale=1.0,
                    )
                # init: P0 *= colmask (bf16 one-hot broadcast is exact for
                # a 0/1 mask), with fused row-sum accumulation
                cb = make_cb(k, cm_d[:], cm_sb, BPC, k)
                for t in range(vt):
                    fused(k, t, 1.0, cb,
                          rs0[:, off[k] + t : off[k] + t + 1])

        for pair in range(NPAIRS):
            last = pair == NPAIRS - 1
            rs_nx = [None] * NG
            for g in range(NG):
                if rs_cur[g] is None:
                    continue
                # row reciprocals for the whole group: rr = 1/(rs + tiny)
                tmp = pool_sm.tile([P, gvt[g]], F32, tag=f"tmp{g}")
                rr = pool_sm.tile([P, gvt[g]], F32, tag=f"rr{g}")
                nc.vector.tensor_scalar_add(tmp, rs_cur[g], TINY)
                nc.vector.reciprocal(rr, tmp)

                if not last:
                    rs_nx[g] = pool_rs.tile([P, gvt[g]], F32, tag=f"rs{g}",
                                            name=f"rsn{pair}_{g}")

                for q4 in range(G // 4):
                    slots = [g * G + q4 * 4 + i for i in range(4)]
                    slots = [k for k in slots if Pt[k] is not None]
                    if not slots:
                        continue
                    # weighted column sums: cs[q, j] = sum_i rr[i] * P[i, j],
                    # 4 slots per PSUM bank at partition offsets 0/32/64/96,
                    # TINY-seeded so fully-masked columns stay finite.
                    csb = psum_cs.tile([P, M], F32, tag="cs")
                    for k in slots:
                        q = k % 4
                        region = csb[32 * q : 32 * q + 1, :]
                        nc.tensor.matmul(
                            out=region[:, : NC[k]], lhsT=tiny1,
                            rhs=ones512[:, : NC[k]],
                            start=True, stop=False, tile_position=(0, 32 * q),
                        )
                        for t in range(VT[k]):
                            nc.tensor.matmul(
                                out=region[:, : NC[k]],
                                lhsT=rr[:, off[k] + t : off[k] + t + 1],
                                rhs=Pt[k][:, t, : NC[k]],
                                start=False, stop=(t == VT[k] - 1),
                                tile_position=(0, 32 * q),
                            )
                    for k in slots:
                        q = k % 4
                        ncw = NC[k]
                        # per-slot single-partition recip: a PSUM read at
                        # partition offset 32q is legal (DVE cannot read
                        # strided-partition views); out lands at partition 0
                        cr1 = pool_sm.tile([1, M], F32, tag=f"cr_{k}",
                                           name=f"cr{pair}_{k}")
                        nc.vector.reciprocal(
                            cr1[:, :ncw], csb[32 * q : 32 * q + 1, :ncw])
                        crd1 = None
                        if CB_MODE == "dma":
                            crd1 = pool_crd.tile([1, M], F32, tag="crd",
                                                 name=f"crd{pair}_{k}")
                            nc.sync.dma_start(out=crd1[:, :ncw],
                                              in_=cr1[:, :ncw])
                        cb = make_cb(k, crd1, cr1, 1, 0)
                        for t in range(VT[k]):
                            acc = None
                            if not last:
                                acc = rs_nx[g][:, off[k] + t : off[k] + t + 1]
                            fused(k, t, rr[:, off[k] + t : off[k] + t + 1],
                                  cb, acc)
            rs_cur = rs_nx

        for k in range(BPC):
            if Pt[k] is not None:
                # valid columns only; the output buffer is pre-zeroed
                nc.sync.dma_start(
                    out=out_all[k][:, : VT[k], : NC[k]],
                    in_=Pt[k][:, :, : NC[k]])

    # Bacc legalization: splits multi-semaphore waits (TRN2 allows one wait
    # per compute instruction), event-semaphore generation, reg alloc, DCE.
    nc.compile()
    return nc


def host_inputs(plan, s, nrows, ncols):
    """Per-core input maps (permuted into slot order) + mask vectors."""
    s = np.ascontiguousarray(np.asarray(s), dtype=np.float32)
    nrows = np.asarray(nrows).astype(np.int64)
    ncols = np.asarray(ncols).astype(np.int64)
    VT, NC = plan["VT"], plan["NC"]
    sample_at = plan["sample_at"]
    sum_vt = max(sum(VT), 1)
    in_maps = []
    for c in range(NCORES):
        idx = sample_at[c]
        s_c = s[idx]
        rb = np.full((P, sum_vt), 0.0, dtype=np.float32)
        pos = 0
        for k in range(BPC):
            nr = nrows[idx[k]]
            for t in range(VT[k]):
                gi = t * P + np.arange(P)
                rb[:, pos] = np.where(gi < nr, 0.0, NEG)
                pos += 1
        cm = (np.arange(M)[None, :] < ncols[idx][:, None]).astype(np.float32)
        in_maps.append(
            {
                "s": np.ascontiguousarray(s_c),
                "rowbias": rb,
                "colmask": np.ascontiguousarray(cm),
            }
        )
    return in_maps


def kernel(s, nrows, ncols):
    global LAST_EXEC_NS
    plan = plan_from_masks(nrows, ncols)
    key = (plan["VT"], plan["NC"], plan["pool_slots"])
    if key not in _CACHE:
        _CACHE[key] = build(plan)
    nc = _CACHE[key]
    in_maps = host_inputs(plan, s, nrows, ncols)
    res = run_bass_kernel_spmd(nc, in_maps, list(range(NCORES)))
    LAST_EXEC_NS = res.exec_time_ns
    out = np.zeros((B, N, M), dtype=np.float32)
    sample_at = plan["sample_at"]
    for c in range(NCORES):
        out[sample_at[c]] = res.results[c]["out"]
    return out
